# revision 1
# baseline (speedup 1.0000x reference)
"""Multi-head attention (no mask) Trainium2 kernel, SPMD over 8 NeuronCores.

Problem: x[2,2048,1024] @ wq/wk/wv[1024,1024] (+zero biases) -> 16-head
scaled-dot-product attention (softmax over full sequence, no causal mask),
output [2,2048,1024] fp32.

Sharding: tensor-parallel over heads. Each core handles 2 heads (128 output
columns) for both batches: per-core weights are the 128-column slice of
wq/wk/wv; per-core output is out[:, :, c0:c0+128]. Host concatenates.

Per-core pipeline (all matmuls bf16 with fp32 PSUM accumulate):
  1. x (fp32 DRAM) --SWDGE cast--> x16 (bf16 DRAM scratch)
  2. x16 --HWDGE DMA transpose--> xT [128 D-part, 2048 s] tiles (8 per batch)
  3. qT/kT/vT [128 (2 heads x 64), 2048] = w_slice.T @ xT  (+bias, cast bf16)
  4. v65: PE-transpose vT into v-natural with a ones column appended
     ([128 kseq, 16 kchunk, 130]: [0:64]=v_h0, [64]=1, [65:129]=v_h1, [129]=1)
  5. per (batch, qchunk of 512): for each kchunk of 128:
       scoresT[k,q] = kT.T @ qT (contraction over head dim, both heads ride
       different PE row-groups), exp on ScalarE (scale=1/8) -> attT bf16,
       yT[65,512] += v65.T @ attT  (row 64 accumulates softmax denominator)
  6. finalize: PE-transpose yT -> [128 q, 65], per-partition reciprocal of
     col 64, multiply, DMA out.
"""

import os
import sys

import numpy as np

for _p in ("/opt/trn_rl_repo", "/root/.axon_site/_ro/trn_rl_repo"):
    if _p not in sys.path and os.path.isdir(_p):
        sys.path.append(_p)

from contextlib import ExitStack

import concourse.bass as bass
import concourse.tile as tile
from concourse import bacc, masks, mybir
from concourse.bass_utils import run_bass_kernel_spmd

FP32 = mybir.dt.float32
BF16 = mybir.dt.bfloat16

N_CORES = 8
B, S, D = 2, 2048, 1024
COLS = 128            # output columns per core = 2 heads x 64
HD = 64               # head dim
SCALE = 0.125         # 1 / sqrt(HD)
QCH = 512             # q chunk (psum free dim)
KCH = 128             # k chunk (partition dim)
NKC = S // KCH        # 16
NQC = S // QCH        # 4
NJ = QCH // 128       # 4 transpose blocks per q chunk
DT = D // 128         # 8 contraction tiles for projections
GROUPS = (2,) * 8             # kchunks per exp block (psum bank budget)
GSLOT = max(GROUPS)

_CACHED_NC = None


def build_nc(reps=1):
    nc = bacc.Bacc("TRN2", target_bir_lowering=False, debug=False,
                   num_devices=N_CORES)

    x = nc.dram_tensor("x", [B, S, D], FP32, kind="ExternalInput").ap()
    w_ap = {}
    b_ap = {}
    for p in ("q", "k", "v"):
        w_ap[p] = nc.dram_tensor(f"w{p}", [D, COLS], FP32,
                                 kind="ExternalInput").ap()
        b_ap[p] = nc.dram_tensor(f"b{p}", [COLS], FP32,
                                 kind="ExternalInput").ap()
    out = nc.dram_tensor("out", [B, S, COLS], FP32, kind="ExternalOutput").ap()

    with tile.TileContext(nc) as tc, ExitStack() as ctx:
        dram_pool = ctx.enter_context(tc.tile_pool(name="dram", bufs=1,
                                                   space="DRAM"))
        const_pool = ctx.enter_context(tc.tile_pool(name="const", bufs=1))
        w_pool = ctx.enter_context(tc.tile_pool(name="w", bufs=1))
        xt_pool = ctx.enter_context(tc.tile_pool(name="xt", bufs=2))
        qkv_pool = ctx.enter_context(tc.tile_pool(name="qkv", bufs=2))
        v65_pool = ctx.enter_context(tc.tile_pool(name="v65", bufs=2))
        att_pool = ctx.enter_context(tc.tile_pool(name="att", bufs=4))
        fin_pool = ctx.enter_context(tc.tile_pool(name="fin", bufs=4))
        yout_pool = ctx.enter_context(tc.tile_pool(name="yout", bufs=2))
        ps_big = ctx.enter_context(tc.tile_pool(name="psbig", bufs=3,
                                                space="PSUM"))
        ps_acc = ctx.enter_context(tc.tile_pool(name="psacc", bufs=2,
                                                space="PSUM"))

        id_f32 = const_pool.tile([128, 128], FP32, tag="idf")
        id_bf16 = const_pool.tile([128, 128], BF16, tag="idb")
        masks.make_identity(nc, id_f32[:])
        masks.make_identity(nc, id_bf16[:])

        # Weights (cast fp32->bf16 during DMA) and biases.
        w_sb = {}
        b_sb = {}
        for p in ("q", "k", "v"):
            wt = w_pool.tile([128, DT, COLS], BF16, tag=f"w{p}")
            for t in range(DT):
                nc.gpsimd.dma_start(out=wt[:, t, :],
                                    in_=w_ap[p][t * 128:(t + 1) * 128, :])
            w_sb[p] = wt
            bt = w_pool.tile([COLS, 1], FP32, tag=f"b{p}")
            nc.sync.dma_start(out=bt[:],
                              in_=b_ap[p].rearrange("(p one) -> p one", one=1))
            b_sb[p] = bt

        # bf16 copies of x in DRAM (enable the 2-byte HWDGE DMA transpose).
        x16 = [dram_pool.tile([S, D], BF16, name=f"x16_{b}")
               for b in range(B)]
        state = {}  # per-rep tiles; cleared each rep

        def emit_cast(b):
            for c in range(4):
                nc.gpsimd.dma_start(
                    out=x16[b][c * 512:(c + 1) * 512, :].rearrange(
                        "s (u v) -> (s u) v", u=4),
                    in_=x[b, c * 512:(c + 1) * 512, :].rearrange(
                        "s (u v) -> (s u) v", u=4))

        def emit_xt(b, half):
            if (b, "xt") not in state:
                state[b, "xt"] = xt_pool.tile([128, DT, S], BF16, tag="xt",
                                              name="xt")
            xt = state[b, "xt"]
            x16v = x16[b].rearrange("(hh s) (t p) -> hh s t p", p=128,
                                    hh=2)
            for t in range(DT):
                nc.sync.dma_start(out=xt[:, t, half * 1024:(half + 1) * 1024],
                                  in_=x16v[half, :, t], transpose=True)

        def emit_proj(b, p, sc):
            if (b, p) not in state:
                state[b, p] = qkv_pool.tile([128, S], BF16, tag=f"{p}T",
                                            name=f"{p}T")
            pt = state[b, p]
            xt = state[b, "xt"]
            ps = ps_big.tile([128, QCH], FP32, tag="big", name="psproj")
            for t in range(DT):
                nc.tensor.matmul(
                    ps[:], lhsT=w_sb[p][:, t, :],
                    rhs=xt[:, t, sc * QCH:(sc + 1) * QCH],
                    start=(t == 0), stop=(t == DT - 1))
            nc.vector.tensor_scalar_add(
                pt[:, sc * QCH:(sc + 1) * QCH], ps[:], b_sb[p][:])

        def emit_v65(b, kc0, n):
            if (b, "v65") not in state:
                v65 = v65_pool.tile([128, NKC, 130], BF16, tag="v65",
                                    name="v65")
                nc.vector.memset(v65[:, :, 64], 1.0)
                nc.vector.memset(v65[:, :, 129], 1.0)
                state[b, "v65"] = v65
            v65 = state[b, "v65"]
            for kc in range(kc0, kc0 + n):
                pvt = ps_big.tile([128, 128], BF16, tag="big", name="psvt")
                nc.tensor.transpose(pvt[:],
                                    state[b, "v"][:, kc * 128:(kc + 1) * 128],
                                    id_bf16[:])
                nc.vector.tensor_copy(v65[:, kc, 0:64], pvt[:, 0:64])
                nc.vector.tensor_copy(v65[:, kc, 65:129], pvt[:, 64:128])

        def emit_attn_qc(b, qc, hook):
            qT, kT, v65 = state[b, "q"], state[b, "k"], state[b, "v65"]
            psy = [ps_acc.tile([65, QCH], FP32, tag="acc", name="psy")
                   for _ in range(2)]
            kc0 = 0
            for g in GROUPS:
                att = []
                for h in range(2):
                    pss = ps_big.tile([128, GSLOT, QCH], FP32, tag="big",
                                      name="pss")
                    for j in range(g):
                        kc = kc0 + j
                        nc.tensor.matmul(
                            pss[:, j, :],
                            lhsT=kT[h * HD:(h + 1) * HD,
                                    kc * 128:(kc + 1) * 128],
                            rhs=qT[h * HD:(h + 1) * HD,
                                   qc * QCH:(qc + 1) * QCH],
                            start=True, stop=True)
                    at = att_pool.tile([128, GSLOT, QCH], BF16, tag="att",
                                       name="att")
                    nc.scalar.activation(
                        at[:, 0:g, :], pss[:, 0:g, :],
                        mybir.ActivationFunctionType.Exp, scale=SCALE)
                    att.append(at)
                for h in range(2):
                    for j in range(g):
                        kc = kc0 + j
                        nc.tensor.matmul(
                            psy[h][:],
                            lhsT=v65[:, kc, h * 65:(h + 1) * 65],
                            rhs=att[h][:, j, :],
                            start=(kc == 0), stop=(kc == NKC - 1))
                kc0 += g
                if hook is not None:
                    hook()

            # Finalize this q chunk: transpose yT -> y, normalize, store.
            yo = yout_pool.tile([128, NJ, COLS], FP32, tag="yo", name="yo")
            for h in range(2):
                ysb = fin_pool.tile([65, QCH], FP32, tag="ysb", name="ysb")
                nc.vector.tensor_copy(ysb[:], psy[h][:])
                for j in range(NJ):
                    pyt = ps_big.tile([128, 65], FP32, tag="big", name="psyt")
                    nc.tensor.transpose(pyt[:], ysb[:, j * 128:(j + 1) * 128],
                                        id_f32[0:65, 0:65])
                    rc = fin_pool.tile([128, 1], FP32, tag="rc", name="rc")
                    nc.vector.reciprocal(rc[:], pyt[:, 64:65])
                    nc.vector.tensor_scalar_mul(
                        yo[:, j, h * HD:(h + 1) * HD], pyt[:, 0:64], rc[:])
            nc.sync.dma_start(
                out=out[b, qc * QCH:(qc + 1) * QCH, :].rearrange(
                    "(j p) c -> p j c", p=128),
                in_=yo[:])

        # Batch 0 prologue, then batch 0 attention with batch 1's
        # prologue interleaved through the scheduler via emission order.
        for _rep in range(reps):
            state.clear()
            emit_cast(0)
            emit_xt(0, 0)
            emit_xt(0, 1)
            for p in ("q", "k", "v"):
                for sc in range(NQC):
                    emit_proj(0, p, sc)
            emit_v65(0, 0, NKC)

            pending = [lambda: emit_cast(1),
                       lambda: emit_xt(1, 0), lambda: emit_xt(1, 1)]
            for p in ("q", "k", "v"):
                for sc in range(NQC):
                    pending.append(lambda p=p, sc=sc: emit_proj(1, p, sc))
            for kc0 in range(0, NKC, 4):
                pending.append(lambda kc0=kc0: emit_v65(1, kc0, 4))
            pending.reverse()

            if os.environ.get("ATTN_NO_INTERLEAVE"):
                while pending:
                    pending.pop()()

            def hook():
                if pending:
                    pending.pop()()

            for qc in range(NQC):
                emit_attn_qc(0, qc, hook)
            while pending:
                pending.pop()()
            for qc in range(NQC):
                emit_attn_qc(1, qc, None)

    nc.compile()
    return nc


def get_nc():
    global _CACHED_NC
    if _CACHED_NC is None:
        _CACHED_NC = build_nc()
    return _CACHED_NC


def make_in_maps(x, wq, bq, wk, bk, wv, bv):
    in_maps = []
    for i in range(N_CORES):
        c0 = i * COLS
        in_maps.append({
            "x": np.ascontiguousarray(x, dtype=np.float32),
            "wq": np.ascontiguousarray(wq[:, c0:c0 + COLS], dtype=np.float32),
            "wk": np.ascontiguousarray(wk[:, c0:c0 + COLS], dtype=np.float32),
            "wv": np.ascontiguousarray(wv[:, c0:c0 + COLS], dtype=np.float32),
            "bq": np.ascontiguousarray(bq[c0:c0 + COLS], dtype=np.float32),
            "bk": np.ascontiguousarray(bk[c0:c0 + COLS], dtype=np.float32),
            "bv": np.ascontiguousarray(bv[c0:c0 + COLS], dtype=np.float32),
        })
    return in_maps


def kernel(x, wq, bq, wk, bk, wv, bv):
    nc = get_nc()
    in_maps = make_in_maps(x, wq, bq, wk, bk, wv, bv)
    res = run_bass_kernel_spmd(nc, in_maps, list(range(N_CORES)))
    parts = [res.results[i]["out"] for i in range(N_CORES)]
    out = np.concatenate(parts, axis=2).astype(np.float32)
    kernel.last_results = res
    return out



# revision 9
# speedup vs baseline: 1.4050x; 1.4050x over previous
"""Multi-head attention (no mask) Trainium2 kernel, SPMD over 8 NeuronCores.

Problem: x[2,2048,1024] @ wq/wk/wv[1024,1024] (+zero biases) -> 16-head
scaled-dot-product attention (softmax over full sequence, no causal mask),
output [2,2048,1024] fp32.

Sharding: batch x head-quad. Core i handles batch i//4 and heads
4*(i%4)..4*(i%4)+4 (256 output columns). Per-core inputs: x[b] [2048,1024],
w*[:, c0:c0+256], b*[c0:c0+256]; per-core output out[2048, 256]. Host
concatenates heads and stacks batches.

Design notes (the scalar engine is the hard floor: ~109us/core of pure exp):
  - everything in the attention inner loop runs in the PE's 64x128 tiling
    mode: score matmuls contract over head_dim=64, so the two heads of a
    pair ride concurrent row-tiles T0/T8; AV matmuls are split into 64-key
    parity halves, also T0/T8 pairs (order T0,T8,T8,T0 per K so the two
    psy accumulator banks never see overlapping write windows).
  - exp consumes a continuous stream of (K, h) score slots packed 3 per
    psum tile [128, 3, 512] (N=1536 per activation, the largest batch that
    still allows double buffering: 2x3 banks + 2 psy banks = 8).
  - softmax denominator rides the AV matmul: v65 col 64 is ones, psy row 64
    accumulates sum(exp); finalize PE-transposes psy, multiplies by the
    reciprocal, and DMAs out. Finalize + projection chunks borrow psum
    slots from the psy/pss pools at block boundaries.
  - x fp32 -> bf16 via SWDGE cast to a DRAM scratch (big contiguous
    descriptors), then HWDGE 2-byte transpose-DMA into xT.
"""

import os
import sys

import numpy as np

for _p in ("/opt/trn_rl_repo", "/root/.axon_site/_ro/trn_rl_repo"):
    if _p not in sys.path and os.path.isdir(_p):
        sys.path.append(_p)

from contextlib import ExitStack

import concourse.bass as bass
import concourse.tile as tile
from concourse import bacc, masks, mybir
from concourse.bass_utils import run_bass_kernel_spmd

FP32 = mybir.dt.float32
BF16 = mybir.dt.bfloat16
Exp = mybir.ActivationFunctionType.Exp

N_CORES = 8
B, S, D = 2, 2048, 1024
COLS = 256            # output columns per core = 4 heads x 64
HP = 2                # head pairs per core
HD = 64               # head dim
SCALE = 0.125         # 1 / sqrt(HD)
QCH = 512             # q chunk (psum free dim)
NKC = 16              # kc128 chunks
NQC = S // QCH        # 4
NJ = QCH // 128       # 4
DT = D // 128         # 8 contraction tiles for projections
GSLOT = 3             # score slots per psum tile / exp instruction

_CACHED_NC = None


def build_nc(reps=1):
    nc = bacc.Bacc("TRN2", target_bir_lowering=False, debug=False,
                   num_devices=N_CORES)

    x = nc.dram_tensor("x", [S, D], FP32, kind="ExternalInput").ap()
    w_ap = {}
    b_ap = {}
    for p in ("q", "k", "v"):
        w_ap[p] = nc.dram_tensor(f"w{p}", [D, COLS], FP32,
                                 kind="ExternalInput").ap()
        b_ap[p] = nc.dram_tensor(f"b{p}", [COLS], FP32,
                                 kind="ExternalInput").ap()
    out = nc.dram_tensor("out", [S, COLS], FP32, kind="ExternalOutput").ap()

    with tile.TileContext(nc) as tc, ExitStack() as ctx:
        dram_pool = ctx.enter_context(tc.tile_pool(name="dram", bufs=1,
                                                   space="DRAM"))
        const_pool = ctx.enter_context(tc.tile_pool(name="const", bufs=1))
        wst_pool = ctx.enter_context(tc.tile_pool(name="wst", bufs=2))
        w_pool = ctx.enter_context(tc.tile_pool(name="w", bufs=1))
        xt_pool = ctx.enter_context(tc.tile_pool(name="xt", bufs=1))
        qkv_pool = ctx.enter_context(tc.tile_pool(name="qkv", bufs=1))
        v65_pool = ctx.enter_context(tc.tile_pool(name="v65", bufs=1))
        att_pool = ctx.enter_context(tc.tile_pool(name="att", bufs=16))
        fin_pool = ctx.enter_context(tc.tile_pool(name="fin", bufs=4))
        yout_pool = ctx.enter_context(tc.tile_pool(name="yout", bufs=2))
        ps_s = ctx.enter_context(tc.tile_pool(name="pss", bufs=2,
                                              space="PSUM"))
        ps_y = ctx.enter_context(tc.tile_pool(name="psy", bufs=2,
                                              space="PSUM"))

        id_f32 = const_pool.tile([128, 128], FP32, tag="idf")
        id_bf16 = const_pool.tile([128, 128], BF16, tag="idb")
        masks.make_identity(nc, id_f32[:])
        masks.make_identity(nc, id_bf16[:])

        x16 = dram_pool.tile([S, D], BF16, name="x16")
        x16v = x16.rearrange("s (t p) -> s t p", p=128)

        state = {}

        def emit_w(p):
            w32 = wst_pool.tile([128, DT, COLS], FP32, tag="wst",
                                name=f"w32{p}")
            nc.scalar.dma_start(out=w32[:],
                                in_=w_ap[p].rearrange("(t p) c -> p t c",
                                                      p=128))
            wt = w_pool.tile([128, DT, COLS], BF16, tag=f"w{p}")
            nc.vector.tensor_copy(wt[:], w32[:])
            state["w", p] = wt
            bt = w_pool.tile([128, HP], FP32, tag=f"b{p}")
            nc.sync.dma_start(out=bt[:],
                              in_=b_ap[p].rearrange("(hp c) -> c hp", c=128))
            state["b", p] = bt

        def emit_cast(sc):
            nc.gpsimd.dma_start(out=x16[sc * QCH:(sc + 1) * QCH, :],
                                in_=x[sc * QCH:(sc + 1) * QCH, :])

        def emit_xt(sc):
            if "xt" not in state:
                state["xt"] = xt_pool.tile([128, DT, S], BF16, tag="xt",
                                           name="xt")
            xt = state["xt"]
            for t in range(DT):
                nc.sync.dma_start(
                    out=xt[:, t, sc * QCH:(sc + 1) * QCH],
                    in_=x16v[sc * QCH:(sc + 1) * QCH, t, :], transpose=True)

        def emit_proj(hp, p, sc):
            if (hp, p) not in state:
                state[hp, p] = qkv_pool.tile([128, S], BF16, tag=f"{p}T{hp}",
                                             name=f"{p}T{hp}")
            pt = state[hp, p]
            xt = state["xt"]
            wt = state["w", p]
            ps = ps_s.tile([128, QCH], FP32, tag="s", name="psproj")
            for t in range(DT):
                nc.tensor.matmul(
                    ps[:], lhsT=wt[:, t, hp * 128:(hp + 1) * 128],
                    rhs=xt[:, t, sc * QCH:(sc + 1) * QCH],
                    start=(t == 0), stop=(t == DT - 1))
            nc.vector.tensor_scalar_add(
                pt[:, sc * QCH:(sc + 1) * QCH], ps[:],
                state["b", p][:, hp:hp + 1])

        def emit_v65(hp, quad):
            if (hp, "v65") not in state:
                v65 = v65_pool.tile([128, NKC, 2, 65], BF16, tag=f"v65{hp}",
                                    name=f"v65{hp}")
                nc.vector.memset(v65[:, :, :, 64], 1.0)
                state[hp, "v65"] = v65
            v65 = state[hp, "v65"]
            vT = state[hp, "v"]
            for K in range(quad * 4, quad * 4 + 4):
                pvt = ps_s.tile([128, 2, 1024], BF16, tag="s", name="psvt")
                for h in range(2):
                    nc.tensor.transpose(
                        pvt[:, h, 0:64],
                        vT[h * HD:(h + 1) * HD, K * 128:(K + 1) * 128],
                        id_bf16[h * HD:(h + 1) * HD, h * HD:(h + 1) * HD])
                for h in range(2):
                    nc.vector.tensor_copy(v65[:, K, h, 0:64], pvt[:, h, 0:64])

        def emit_finalize_h(hp, qc, h, psy, yo):
            # merge the two parity banks, transpose, normalize into yo
            ysb = fin_pool.tile([65, QCH], FP32, tag="ysb", name="ysb")
            nc.vector.tensor_copy(ysb[:], psy[0][:])
            nc.vector.scalar_tensor_tensor(
                ysb[:], psy[1][:], 0.0, ysb[:],
                mybir.AluOpType.add, mybir.AluOpType.add)
            for jj in range(NJ):
                pyt = ps_y.tile([128, 65], FP32, tag="y", name="psyt")
                nc.tensor.transpose(pyt[:],
                                    ysb[:, jj * 128:(jj + 1) * 128],
                                    id_f32[0:65, 0:65])
                rc = fin_pool.tile([128, 1], FP32, tag="rc", name="rc")
                nc.vector.reciprocal(rc[:], pyt[:, 64:65])
                nc.vector.tensor_scalar_mul(
                    yo[:, jj, h * HD:(h + 1) * HD], pyt[:, 0:64], rc[:])
            if h == 1:
                nc.sync.dma_start(
                    out=out[qc * QCH:(qc + 1) * QCH,
                            hp * 128:(hp + 1) * 128].rearrange(
                        "(j p) c -> p j c", p=128),
                    in_=yo[:])

        def run_rep():
            state.clear()
            for p in ("q", "k", "v"):
                emit_w(p)
            for sc in range(NQC):
                emit_cast(sc)
            for sc in range(NQC):
                emit_xt(sc)
            emit_proj(0, "k", 0)
            emit_proj(0, "q", 0)
            emit_proj(0, "v", 0)
            emit_proj(0, "k", 1)
            emit_proj(0, "v", 1)
            emit_v65(0, 0)
            emit_v65(0, 1)

            pending = [
                lambda: emit_proj(0, "k", 2), lambda: emit_proj(0, "v", 2),
                lambda: emit_v65(0, 2),
                lambda: emit_proj(0, "k", 3), lambda: emit_proj(0, "v", 3),
                lambda: emit_v65(0, 3),
                lambda: emit_proj(0, "q", 1),
                lambda: emit_proj(1, "k", 0), lambda: emit_proj(1, "v", 0),
                lambda: emit_v65(1, 0),
                lambda: emit_proj(1, "k", 1), lambda: emit_proj(1, "v", 1),
                lambda: emit_v65(1, 1),
                lambda: emit_proj(1, "q", 0),
                lambda: emit_proj(1, "k", 2), lambda: emit_proj(1, "v", 2),
                lambda: emit_v65(1, 2),
                lambda: emit_proj(1, "k", 3), lambda: emit_proj(1, "v", 3),
                lambda: emit_v65(1, 3),
                lambda: emit_proj(0, "q", 2),
                lambda: emit_proj(1, "q", 1),
                lambda: emit_proj(0, "q", 3),
                lambda: emit_proj(1, "q", 2),
                lambda: emit_proj(1, "q", 3),
            ]
            pending.reverse()

            def hook():
                if pending:
                    pending.pop()()

            # --- slot stream -------------------------------------------------
            # slots iterate (hp, qc, K, h) with h innermost so score matmuls
            # alternate PE row-tiles T0/T8 (concurrent pairs). AV runs in
            # h-phases: h0's AVs go out as soon as att is ready; h1's are
            # deferred (att tiles held in SBUF) until h0's two parity psum
            # banks are finalized, so each psy bank is only ever written by
            # one row-tile (T0 -> par0 bank, T8 -> par1 bank).
            stream = [(hp, qc, K, h)
                      for hp in range(HP) for qc in range(NQC)
                      for K in range(NKC) for h in range(2)]

            att_of = {}
            cur = {"tile": None, "at_slots": []}
            phase = {"key": None, "psy": None, "fin": None}
            blockstate = {}

            def get_bs(hp, qc):
                bs = blockstate.get((hp, qc))
                if bs is None:
                    bs = {"h0_done": False, "deferred": [],
                          "yo": yout_pool.tile([128, NJ, 128], FP32,
                                               tag="yo", name="yo")}
                    blockstate[(hp, qc)] = bs
                return bs

            def do_av(hp, qc, h, K):
                if phase["key"] != (hp, qc, h):
                    if phase["fin"] is not None:
                        phase["fin"]()
                    phase["key"] = (hp, qc, h)
                    phase["psy"] = [ps_y.tile([65, QCH], FP32, tag="y",
                                              name=f"psy{par}")
                                    for par in range(2)]
                    psy_now = phase["psy"]
                    yo_now = get_bs(hp, qc)["yo"]
                    phase["fin"] = (lambda hp=hp, qc=qc, h=h, psy=psy_now,
                                    yo=yo_now:
                                    emit_finalize_h(hp, qc, h, psy, yo))
                psy = phase["psy"]
                v65 = state[hp, "v65"]
                a, i = att_of.pop((hp, qc, h, K))
                for par in range(2):
                    nc.tensor.matmul(
                        psy[par][:],
                        lhsT=v65[par * 64:(par + 1) * 64, K, h, :],
                        rhs=a[par * 64:(par + 1) * 64, i, :],
                        start=(K == 0), stop=(K == NKC - 1))

            def flush():
                n = len(cur["at_slots"])
                if n == 0:
                    return
                at = att_pool.tile([128, GSLOT, QCH], BF16, tag="att",
                                   name="att")
                nc.scalar.activation(at[:, 0:n, :], cur["tile"][:, 0:n, :],
                                     Exp, scale=SCALE)
                slots = cur["at_slots"]
                cur["tile"] = None
                cur["at_slots"] = []
                for idx, (hp, qc, K, h) in enumerate(slots):
                    att_of[(hp, qc, h, K)] = (at, idx)
                    bs = get_bs(hp, qc)
                    if h == 0:
                        do_av(hp, qc, 0, K)
                        hook()
                        if K == NKC - 1:
                            bs["h0_done"] = True
                            for K1 in bs["deferred"]:
                                do_av(hp, qc, 1, K1)
                                hook()
                            bs["deferred"] = []
                    else:
                        if bs["h0_done"]:
                            do_av(hp, qc, 1, K)
                            hook()
                        else:
                            bs["deferred"].append(K)

            for slot in stream:
                hp, qc, K, h = slot
                if cur["tile"] is None:
                    cur["tile"] = ps_s.tile([128, GSLOT, QCH], FP32, tag="s",
                                            name="pss")
                idx = len(cur["at_slots"])
                nc.tensor.matmul(
                    cur["tile"][:, idx, :],
                    lhsT=state[hp, "k"][h * HD:(h + 1) * HD,
                                        K * 128:(K + 1) * 128],
                    rhs=state[hp, "q"][h * HD:(h + 1) * HD,
                                       qc * QCH:(qc + 1) * QCH],
                    start=True, stop=True)
                cur["at_slots"].append(slot)
                if len(cur["at_slots"]) == GSLOT:
                    flush()
            flush()
            while pending:
                pending.pop()()
            phase["fin"]()

        for _rep in range(reps):
            run_rep()

    nc.compile()
    return nc


def get_nc():
    global _CACHED_NC
    if _CACHED_NC is None:
        _CACHED_NC = build_nc()
    return _CACHED_NC


def make_in_maps(x, wq, bq, wk, bk, wv, bv):
    in_maps = []
    for i in range(N_CORES):
        b = i // 4
        c0 = (i % 4) * COLS
        in_maps.append({
            "x": np.ascontiguousarray(x[b], dtype=np.float32),
            "wq": np.ascontiguousarray(wq[:, c0:c0 + COLS], dtype=np.float32),
            "wk": np.ascontiguousarray(wk[:, c0:c0 + COLS], dtype=np.float32),
            "wv": np.ascontiguousarray(wv[:, c0:c0 + COLS], dtype=np.float32),
            "bq": np.ascontiguousarray(bq[c0:c0 + COLS], dtype=np.float32),
            "bk": np.ascontiguousarray(bk[c0:c0 + COLS], dtype=np.float32),
            "bv": np.ascontiguousarray(bv[c0:c0 + COLS], dtype=np.float32),
        })
    return in_maps


def assemble(res, inputs=None):
    batches = []
    for b in range(B):
        parts = [res.results[b * 4 + q]["out"] for q in range(4)]
        batches.append(np.concatenate(parts, axis=1))
    return np.stack(batches).astype(np.float32)


def kernel(x, wq, bq, wk, bk, wv, bv):
    nc = get_nc()
    in_maps = make_in_maps(x, wq, bq, wk, bk, wv, bv)
    res = run_bass_kernel_spmd(nc, in_maps, list(range(N_CORES)))
    out = assemble(res)
    kernel.last_results = res
    return out
